# revision 1
# baseline (speedup 1.0000x reference)
"""EntityAttentionLayer Trainium2 kernel.

Data-parallel over batch across 8 NeuronCores (256 batches/core).
Per core, per G-group of 16 batches (1024 entity tokens, 256 query tokens):
  - XT:   entities loaded feature-on-partition via strided DMA  [128, 4dc, 1024]
  - K:    feature-major GEMM   kf [128, 4ec, 1024]   (e-chunk c = heads 2c,2c+1)
  - V:    token-major GEMM     vt [128, 8tc, 512]    (partitions = tokens)
  - Q:    gather q-tokens from XT, feature-major GEMM qf [128, 4ec, 256]
  - BDQ:  block-diagonal Q  [128=(2h x 64d), 4c, 16b, 32=(2h x 16q)]
  - logits^T via PE: psum_l [64j, 16b, 128=(4c x 2h x 16q)]
  - mask add (-1e30), exp via ACT (scale=1/8) -> BDE [128=(2b x 64j), 4c, 8g2, 64]
  - sums via ones-matmul -> psum_s[0], reciprocal (ACT) -> rs_sb
  - attn@V per (g2, c): psum_av [128feat, (4 x 256)]
  - extract diag blocks -> ao [128, 4ec, 256], scale by rs (broadcast DMA)
  - out-proj GEMM -> psum_o [128tok, 512], post-mask multiply, DMA out
"""

import os
import sys

import numpy as np

sys.path.insert(0, "/opt/trn_rl_repo")

import concourse.bass as bass
import concourse.mybir as mybir
import concourse.tile as tile
from concourse import bacc, bass_utils

F32 = mybir.dt.float32
F32R = mybir.dt.float32r
AF = mybir.ActivationFunctionType
ALU = mybir.AluOpType

BS, NE, NQ = 2048, 64, 16
D = 512
H = 8
HD = 64
NCORES = 8
BPC = BS // NCORES          # 256 batches per core
GB = 16                     # batches per G-group
NG = BPC // GB              # 16 groups
NTOK = BPC * NE             # 16384 entity tokens per core
NQT = BPC * NQ              # 4096 query tokens per core

# which matmul sites use float32r (fast fp32) — tuned after accuracy check
F32R_SITES = set(os.environ.get("F32R_SITES", "").split(",")) - {""}


def _mm(nc, out, lhsT, rhs, site, **kw):
    if site in F32R_SITES:
        lhsT = lhsT.bitcast(F32R)
        rhs = rhs.bitcast(F32R)
    nc.tensor.matmul(out, lhsT, rhs, **kw)


def build_nc(debug=False):
    nc = bacc.Bacc()
    dbg = {}
    if debug:
        for nm, shp in [
            ("dxt", [128, 4096]), ("dqf", [128, 1024]), ("dkf", [128, 4096]),
            ("dvt", [128, 4096]), ("dbde", [128, 2048]), ("drs", [1, 2048]),
            ("dao", [128, 1024]), ("dbdq", [128, 2048]),
            ("drf", [128, 1024]), ("dex", [128, 1024]),
        ]:
            dbg[nm] = nc.declare_dram_parameter(nm, shp, F32, isOutput=True)

    ent = nc.declare_dram_parameter("ent", [D, NTOK], F32R, isOutput=False)
    mneg = nc.declare_dram_parameter("mneg", [NE, NQT], F32, isOutput=False)
    pmt = nc.declare_dram_parameter("pmt", [NQT], F32, isOutput=False)
    wqT = nc.declare_dram_parameter("wqT", [D, 512], F32R, isOutput=False)
    wkT = nc.declare_dram_parameter("wkT", [D, 512], F32R, isOutput=False)
    wvT = nc.declare_dram_parameter("wvT", [D, 512], F32R, isOutput=False)
    woT = nc.declare_dram_parameter("woT", [512, 512], F32R, isOutput=False)
    out = nc.declare_dram_parameter("out", [NQT, 512], F32, isOutput=True)

    ent_r = ent.rearrange("(dc p) n -> p dc n", p=128)   # [128, 4, 16384]
    mneg_r = mneg.rearrange("j (b q) -> j b q", q=NQ)    # [64, 256, 16]
    pmt_r = pmt.rearrange("(o p) -> p o", p=128)         # [128, 32]

    with tile.TileContext(nc) as tc:
        with (
            tc.tile_pool(name="wpool", bufs=1) as wpool,
            tc.tile_pool(name="xtp", bufs=2) as xtp,
            tc.tile_pool(name="kfp", bufs=2) as kfp,
            tc.tile_pool(name="vtp", bufs=1) as vtp,
            tc.tile_pool(name="xqp", bufs=2) as xqp,
            tc.tile_pool(name="qfp", bufs=2) as qfp,
            tc.tile_pool(name="mnp", bufs=2) as mnp,
            tc.tile_pool(name="rsp", bufs=1) as rsp,
            tc.tile_pool(name="rfp", bufs=2) as rfp,
            tc.tile_pool(name="aop", bufs=2) as aop,
            tc.tile_pool(name="osp", bufs=2) as osp,
            tc.tile_pool(name="persist", bufs=1) as persist,
            tc.tile_pool(name="ps", bufs=2, space="PSUM") as psp,
            tc.tile_pool(name="psl", bufs=1, space="PSUM") as pslp,
            tc.tile_pool(name="avp", bufs=1, space="PSUM") as avp,
            tc.tile_pool(name="dmp", bufs=1, space="PSUM") as dmp,
        ):
            # ---- constants / weights (loaded once) ----
            wq_t = wpool.tile([128, 4, 512], F32R, tag="wq")
            wk_t = wpool.tile([128, 4, 512], F32R, tag="wk")
            wv_t = wpool.tile([128, 4, 512], F32R, tag="wv")
            wo_t = wpool.tile([128, 4, 512], F32R, tag="wo")
            nc.sync.dma_start(wq_t, wqT.rearrange("(dc p) e -> p dc e", p=128))
            nc.sync.dma_start(wk_t, wkT.rearrange("(dc p) e -> p dc e", p=128))
            nc.sync.dma_start(wv_t, wvT.rearrange("(dc p) e -> p dc e", p=128))
            nc.sync.dma_start(wo_t, woT.rearrange("(ec p) o -> p ec o", p=128))
            pm_t = wpool.tile([128, 32], F32, tag="pm")
            nc.sync.dma_start(pm_t, pmt_r)
            ones_t = wpool.tile([128, 1], F32, tag="ones")
            nc.vector.memset(ones_t, 1.0)
            fence_t = wpool.tile([1, 4], F32, tag="fence")
            nc.vector.memset(fence_t, 0.0)

            # persistent block-diagonal buffers (off-blocks stay zero forever)
            bdqs = [
                persist.tile([128, 4, GB, 32], F32, tag=f"bdq{i}", name=f"bdq{i}")
                for i in range(2)
            ]
            bdes = [
                persist.tile([128, 4, 8, 64], F32, tag=f"bde{i}", name=f"bde{i}")
                for i in range(2)
            ]
            for t in bdqs + bdes:
                nc.vector.memset(t, 0.0)

            # scratch psum for wait-absorber dummy matmuls (walrus allows only
            # one semaphore wait on a self-loading fp32 matmul, so dummies
            # pre-absorb init-DMA / cross-engine sem values into PE's clock)
            dummy_ps = dmp.tile([1, 16], F32, tag="dummy")
            for i, t in enumerate([wq_t, wk_t, wv_t, wo_t, ones_t]):
                corner = t[0:1, 0, 0:1] if len(t.shape) == 3 else t[0:1, 0:1]
                corner = corner.bitcast(F32)
                nc.tensor.matmul(
                    dummy_ps[0:1, i : i + 1], corner, corner,
                    start=True, stop=True,
                )

            prev_outs = None
            for g in range(NG):
                bdq = bdqs[g % 2]
                bde = bdes[g % 2]
                tok0 = g * GB * NE          # 1024 entity tokens per group
                q0 = g * GB * NQ            # 256 query tokens per group

                # absorber: advance PE's DVE clock past prior-G tail writes
                absorb = ones_t if prev_outs is None else prev_outs
                nc.tensor.matmul(
                    dummy_ps[0:1, 5:6], absorb[0:1, 0:1], absorb[0:1, 0:1],
                    start=True, stop=True,
                )

                # ---- load entities transposed: [128 d, 4 dc, 1024 tok] ----
                xt = xtp.tile([128, 4, GB * NE], F32R, tag="xt")
                nc.sync.dma_start(xt, ent_r[:, :, tok0 : tok0 + GB * NE])

                # ---- K feature-major GEMM ----
                kf = kfp.tile([128, 4, GB * NE], F32, tag="kf")
                for ec in range(4):
                    for fg in range(2):
                        ps_k = psp.tile([128, 512], F32, tag="ps", name="ps_k")
                        for dc in range(4):
                            _mm(
                                nc, ps_k,
                                wk_t[:, dc, ec * 128 : (ec + 1) * 128],
                                xt[:, dc, fg * 512 : (fg + 1) * 512],
                                "kproj", start=(dc == 0), stop=(dc == 3),
                            )
                        nc.vector.tensor_copy(kf[:, ec, fg * 512 : (fg + 1) * 512], ps_k)

                # ---- Q: gather q-tokens, feature-major GEMM ----
                xq = xqp.tile([128, 4, GB * NQ], F32R, tag="xq")
                nc.vector.tensor_copy(
                    xq.rearrange("p dc (b q) -> p dc b q", b=GB),
                    xt.rearrange("p dc (b t) -> p dc b t", b=GB)[:, :, :, 0:NQ],
                )
                qf = qfp.tile([128, 4, GB * NQ], F32, tag="qf")
                for ec in range(4):
                    ps_q = psp.tile([128, 256], F32, tag="ps", name="ps_q")
                    for dc in range(4):
                        _mm(
                            nc, ps_q,
                            wq_t[:, dc, ec * 128 : (ec + 1) * 128],
                            xq[:, dc, :],
                            "qproj", start=(dc == 0), stop=(dc == 3),
                        )
                    nc.vector.tensor_copy(qf[:, ec, :], ps_q)

                # ---- V token-major GEMM (psum->sbuf copies on ACT) ----
                vt = vtp.tile([128, 8, 512], F32, tag="vt")
                for tc8 in range(8):
                    ps_v = psp.tile([128, 512], F32, tag="ps", name="ps_v")
                    for dc in range(4):
                        _mm(
                            nc, ps_v,
                            xt[:, dc, tc8 * 128 : (tc8 + 1) * 128],
                            wv_t[:, dc, :],
                            "vproj", start=(dc == 0), stop=(dc == 3),
                        )
                    nc.scalar.copy(vt[:, tc8, :], ps_v)

                skip_attn = os.environ.get("SKIP_ATTN") == "1"
                # ---- BDQ build (in-blocks only; off-blocks persist zero) ----
                qf_v = qf.rearrange("p c (b q) -> p c b q", b=GB)
                if skip_attn:
                    ao = aop.tile([128, 4, GB * NQ], F32R, tag="ao")
                    nc.vector.tensor_copy(ao, qf.bitcast(F32R))
                    prev_outs = None
                if not skip_attn:
                    nc.vector.tensor_copy(bdq[0:64, :, :, 0:16], qf_v[0:64])
                    nc.vector.tensor_copy(bdq[64:128, :, :, 16:32], qf_v[64:128])

                    # ---- mask tile ----
                    mn = mnp.tile([64, GB, NQ], F32, tag="mn")
                    nc.sync.dma_start(mn, mneg_r[:, g * GB : (g + 1) * GB, :])

                    # ---- logits^T: psum_l[j, b, (c,2h,16q)] ----
                    ps_l = pslp.tile([64, GB, 128], F32, tag="psl", name="ps_l")
                    for b in range(GB):
                        for c in range(4):
                            _mm(
                                nc, ps_l[:, b, c * 32 : (c + 1) * 32],
                                kf[:, c, b * 64 : (b + 1) * 64],
                                bdq[:, c, b, :],
                                "logits", start=True, stop=True,
                            )
                    # additive mask, broadcast over (c, h-parity)
                    ps_lv = ps_l.rearrange("p b (c x) -> p b c x", c=4)
                    for x0 in (0, 16):
                        nc.vector.tensor_tensor(
                            ps_lv[:, :, :, x0 : x0 + 16],
                            ps_lv[:, :, :, x0 : x0 + 16],
                            mn[:, :, None, :].broadcast_to((64, GB, 4, 16)),
                            ALU.add,
                        )
                    # fence: cells from both mask-written regions, so a dummy
                    # matmul can absorb the mask-DVE sem value into PE's clock
                    nc.vector.tensor_copy(fence_t[0:1, 0:1], ps_l[0:1, 0, 0:1])
                    nc.vector.tensor_copy(fence_t[0:1, 1:2], ps_l[0:1, 0, 16:17])

                    # ---- exp -> BDE in-blocks ----
                    ps_le = ps_l.rearrange("p (g2 h) (c x) -> p g2 h c x", h=2, c=4)
                    for half in (0, 1):
                        nc.scalar.activation(
                            bde[half * 64 : (half + 1) * 64]
                            .rearrange("p c g x -> p g c x")[:, :, :, half * 32 : half * 32 + 32],
                            ps_le[:, :, half, :, :],
                            AF.Exp,
                            scale=1.0 / np.sqrt(HD),
                        )

                    # absorber for mask-DVE values before sums/attnV WAR reuse
                    nc.tensor.matmul(
                        dummy_ps[0:1, 6:8], fence_t[0:1, 0:1], fence_t[0:1, 0:2],
                        start=True, stop=True,
                    )

                    # ---- sums (ones-matmul) + reciprocal ----
                    ps_s = pslp.tile([2, 2048], F32, tag="psl", name="ps_s")
                    for c in range(4):
                        for g2 in range(8):
                            _mm(
                                nc, ps_s[0:1, c * 512 + g2 * 64 : c * 512 + (g2 + 1) * 64],
                                ones_t,
                                bde[:, c, g2, :],
                                "sums", start=True, stop=True,
                            )
                    rs_sb = rsp.tile([1, 2048], F32, tag="rs")
                    nc.vector.tensor_scalar(rs_sb, ps_s[0:1, :], 1e-30, None, ALU.max)
                    nc.vector.reciprocal(rs_sb, rs_sb)

                    # ---- attn @ V ----
                    ao = aop.tile([128, 4, GB * NQ], F32R, tag="ao")
                    for pair in range(4):
                        ps_av = avp.tile([128, 512], F32, tag="av", name="ps_av")
                        for s in range(2):
                            g2 = pair * 2 + s
                            for c in range(4):
                                _mm(
                                    nc, ps_av[:, s * 256 + c * 64 : s * 256 + (c + 1) * 64],
                                    vt[:, g2, c * 128 : (c + 1) * 128],
                                    bde[:, c, g2, :],
                                    "attnv", start=True, stop=True,
                                )
                        # extract diagonal (head-parity) blocks -> ao
                        for P in (0, 1):
                            for B in (0, 1):
                                src = ps_av.rearrange("p (s c y) -> p s c y", s=2, c=4)[
                                    P * 64 : (P + 1) * 64, :, :,
                                    B * 32 + P * 16 : B * 32 + P * 16 + 16,
                                ]
                                dst = ao[P * 64 : (P + 1) * 64].rearrange(
                                    "p c (g2 y) -> p g2 c y", g2=8
                                )[:, pair * 2 : pair * 2 + 2, :, B * 16 : B * 16 + 16]
                                nc.vector.tensor_copy(dst, src)

                    # ---- normalize: ao *= rs (gpsimd partition broadcast) ----
                    if debug and g == 0:
                        nc.sync.dma_start(
                            dbg["dex"][:, :], ao.rearrange("p a b -> p (a b)").bitcast(F32)
                        )
                    # gpsimd partition_broadcast writes lanes [0, channels) only —
                    # it ignores the dst base partition — so build one full-128
                    # broadcast tile per head parity and scale each row-half.
                    rs_fs = []
                    for F in (0, 1):
                        rf = rfp.tile([128, 4, GB * NQ], F32, tag=f"rsf{F}", name=f"rf{F}")
                        for half in (0, 1):
                            dst = rf.rearrange("p c (g2 y) -> p c g2 y", g2=8)[
                                :, :, :, half * 16 : half * 16 + 16
                            ]
                            src = rs_sb.rearrange(
                                "p (c g2 h x) -> p c g2 h x", g2=8, c=4, h=2
                            )[:, :, :, half, F * 16 : F * 16 + 16]
                            nc.gpsimd.partition_broadcast(dst, src, channels=128)
                        rs_fs.append(rf)
                    if debug and g == 0:
                        nc.sync.dma_start(
                            dbg["drf"][0:64, :], rs_fs[0][0:64].rearrange("p a b -> p (a b)")
                        )
                        nc.sync.dma_start(
                            dbg["drf"][64:128, :],
                            rs_fs[1][64:128].rearrange("p a b -> p (a b)"),
                        )
                    nc.vector.tensor_tensor(ao[0:64], ao[0:64], rs_fs[0][0:64], ALU.mult)
                    nc.vector.tensor_tensor(
                        ao[64:128], ao[64:128], rs_fs[1][64:128], ALU.mult
                    )

                if debug and g == 0:
                    nc.sync.dma_start(dbg["dxt"][:, :], xt.rearrange("p a b -> p (a b)").bitcast(F32))
                    nc.sync.dma_start(dbg["dqf"][:, :], qf.rearrange("p a b -> p (a b)"))
                    nc.sync.dma_start(dbg["dkf"][:, :], kf.rearrange("p a b -> p (a b)"))
                    nc.sync.dma_start(dbg["dvt"][:, :], vt.rearrange("p a b -> p (a b)"))
                    nc.sync.dma_start(
                        dbg["dbde"][:, :], bde.rearrange("p a b c -> p (a b c)")
                    )
                    nc.sync.dma_start(dbg["drs"][:, :], rs_sb)
                    nc.sync.dma_start(dbg["dao"][:, :], ao.rearrange("p a b -> p (a b)").bitcast(F32))
                    nc.sync.dma_start(
                        dbg["dbdq"][:, :], bdq.rearrange("p a b c -> p (a b c)")
                    )

                # ---- output projection + post-mask ----
                for tc2 in range(2):
                    ps_o = psp.tile([128, 512], F32, tag="ps", name="ps_o")
                    for ec in range(4):
                        _mm(
                            nc, ps_o,
                            ao[:, ec, tc2 * 128 : (tc2 + 1) * 128],
                            wo_t[:, ec, :],
                            "oproj", start=(ec == 0), stop=(ec == 3),
                        )
                    out_s = osp.tile([128, 512], F32, tag="outs", name="out_s")
                    nc.vector.tensor_scalar(
                        out_s, ps_o, pm_t[:, g * 2 + tc2 : g * 2 + tc2 + 1], None, ALU.mult
                    )
                    nc.sync.dma_start(
                        out[q0 + tc2 * 128 : q0 + (tc2 + 1) * 128, :], out_s
                    )
                    prev_outs = out_s

    nc.finalize()
    return nc


_NC_CACHE = None
RUN_KWARGS = {}
LAST_RESULT = None


def _get_nc():
    global _NC_CACHE
    if _NC_CACHE is None:
        _NC_CACHE = build_nc()
    return _NC_CACHE


def kernel(entities, pre_mask, post_mask, W_in, W_out, b_out):
    entities = np.asarray(entities, dtype=np.float32)
    pre_mask = np.asarray(pre_mask)
    post_mask = np.asarray(post_mask)
    W_in = np.asarray(W_in, dtype=np.float32)
    W_out = np.asarray(W_out, dtype=np.float32)
    b_out = np.asarray(b_out, dtype=np.float32)

    wqT = np.ascontiguousarray(W_in[0:512].T)
    wkT = np.ascontiguousarray(W_in[512:1024].T)
    wvT = np.ascontiguousarray(W_in[1024:1536].T)
    woT = np.ascontiguousarray(W_out.T)

    in_maps = []
    for i in range(NCORES):
        bsl = slice(i * BPC, (i + 1) * BPC)
        ent_i = np.ascontiguousarray(entities[bsl].reshape(NTOK, D).T)
        pm_i = pre_mask[bsl, :NQ, :]  # (256, 16, 64) True -> masked
        mneg_i = np.ascontiguousarray(
            (pm_i.astype(np.float32) * -1e30).transpose(2, 0, 1).reshape(NE, NQT)
        )
        pmt_i = np.ascontiguousarray(
            (1.0 - post_mask[bsl].astype(np.float32)).reshape(NQT)
        )
        in_maps.append(
            {
                "ent": ent_i,
                "mneg": mneg_i,
                "pmt": pmt_i,
                "wqT": wqT,
                "wkT": wkT,
                "wvT": wvT,
                "woT": woT,
            }
        )

    nc = _get_nc()
    res = bass_utils.run_bass_kernel_spmd(
        nc, in_maps, list(range(NCORES)), **RUN_KWARGS
    )
    global LAST_RESULT
    LAST_RESULT = res
    outs = [res.results[i]["out"].reshape(BPC, NQ, 512) for i in range(NCORES)]
    full = np.concatenate(outs, axis=0)
    if b_out.any():
        full = full + b_out[None, None, :]
        full = np.where(post_mask[:, :, None], 0.0, full)
    return full.astype(np.float32)



# revision 11
# speedup vs baseline: 3.1764x; 3.1764x over previous
"""EntityAttentionLayer Trainium2 kernel (v2, all-bf16 datapath).

Data-parallel over batch across 8 NeuronCores (256 batches/core).
Per core, per G-group of 16 batches (1024 entity tokens, 256 query tokens),
software-pipelined so group g's attention runs while group g+1's K/Q
projections keep the PE busy:

  xt:    entities, feature-on-partition, bf16        [128, 4dc, 1024]
  kf:    K feature-major GEMM -> bf16                [128, 4c, 1024]
  vt:    V token-major GEMM -> bf16                  [128, 8t, 512]
  bdq:   Q gathered from xt via strided moving AP, written block-diagonal
         over head-parity                            [128, 4c, 8g2, 64]
  logits: per (c, g2): kf[128,128] stationary (2 batches) x bdq cols
         -> psum [128 tok, 8g2, 64]; junk cross-batch cells + pre-mask
         handled by ONE identity-stationary matmul adding mneg (-1e30)
  exp:   ACT -> bde bf16 (masked cells exp -> 0)     [128, 4c, 8g2, 64]
  sums:  all-ones [128,128] stationary matmul -> denominators replicated
         across all 128 partitions; reciprocal_approx_fast -> rs f32
  attnv: vt stationary x bde moving -> psum with junk parity blocks;
         diagonal blocks extracted * rs (fused normalize) -> ao bf16
  oproj: ao stationary x woT moving; post-mask applied via ACT Copy with
         per-partition scale; DMA out f32
"""

import os
import sys

import numpy as np

sys.path.insert(0, "/opt/trn_rl_repo")

import concourse.bass as bass
import concourse.mybir as mybir
import concourse.tile as tile
from concourse import bacc, bass_utils

import ml_dtypes

F32 = mybir.dt.float32
BF16 = mybir.dt.bfloat16
AF = mybir.ActivationFunctionType
ALU = mybir.AluOpType

BS, NE, NQ = 2048, 64, 16
D = 512
H = 8
HD = 64
NCORES = 8
BPC = BS // NCORES          # 256 batches per core
GB = 16                     # batches per G-group
NG = BPC // GB              # 16 groups
NTOK = BPC * NE             # 16384 entity tokens per core
NQT = BPC * NQ              # 4096 query tokens per core


def build_nc(debug=False):
    nc = bacc.Bacc()
    dbg = {}
    if debug:
        for nm, shp, dt in [
            ("dxt", [128, 4096], BF16), ("dkf", [128, 4096], BF16),
            ("dbdq", [128, 2048], BF16), ("dbde", [128, 2048], BF16),
            ("dvt", [128, 4096], BF16), ("drs", [128, 2048], F32),
            ("dao", [128, 1024], BF16),
        ]:
            dbg[nm] = nc.declare_dram_parameter(nm, shp, dt, isOutput=True)

    ent = nc.declare_dram_parameter("ent", [D, NTOK], BF16, isOutput=False)
    mneg = nc.declare_dram_parameter("mneg", [128, NG * 512], BF16, isOutput=False)
    pmt = nc.declare_dram_parameter("pmt", [NQT], F32, isOutput=False)
    wqT = nc.declare_dram_parameter("wqT", [D, 512], BF16, isOutput=False)
    wkT = nc.declare_dram_parameter("wkT", [D, 512], BF16, isOutput=False)
    wvT = nc.declare_dram_parameter("wvT", [D, 512], BF16, isOutput=False)
    woT = nc.declare_dram_parameter("woT", [512, 512], BF16, isOutput=False)
    idn = nc.declare_dram_parameter("idn", [128, 128], BF16, isOutput=False)
    out = nc.declare_dram_parameter("out", [NQT, 512], F32, isOutput=True)

    ent_r = ent.rearrange("(dc p) n -> p dc n", p=128)   # [128, 4, 16384]
    pmt_r = pmt.rearrange("(o p) -> p o", p=128)         # [128, 32]

    with tile.TileContext(nc) as tc:
        with (
            tc.tile_pool(name="wpool", bufs=1) as wpool,
            tc.tile_pool(name="xtp", bufs=2) as xtp,
            tc.tile_pool(name="kfp", bufs=2) as kfp,
            tc.tile_pool(name="vtp", bufs=2) as vtp,
            tc.tile_pool(name="persist", bufs=1) as persist,
            tc.tile_pool(name="bdep", bufs=2) as bdep,
            tc.tile_pool(name="mnp", bufs=2) as mnp,
            tc.tile_pool(name="rsp", bufs=2) as rsp,
            tc.tile_pool(name="aop", bufs=2) as aop,
            tc.tile_pool(name="osp", bufs=2) as osp,
            tc.tile_pool(name="psp", bufs=2, space="PSUM") as psp,
            tc.tile_pool(name="psqp", bufs=1, space="PSUM") as psqp,
            tc.tile_pool(name="pslp", bufs=2, space="PSUM") as pslp,
            tc.tile_pool(name="avp", bufs=1, space="PSUM") as avp,
        ):
            # ---- constants / weights (loaded once) ----
            wq_t = wpool.tile([128, 4, 512], BF16, tag="wq")
            wk_t = wpool.tile([128, 4, 512], BF16, tag="wk")
            wv_t = wpool.tile([128, 4, 512], BF16, tag="wv")
            wo_t = wpool.tile([128, 4, 512], BF16, tag="wo")
            nc.sync.dma_start(wk_t, wkT.rearrange("(dc p) e -> p dc e", p=128))
            nc.sync.dma_start(wq_t, wqT.rearrange("(dc p) e -> p dc e", p=128))
            nc.sync.dma_start(wv_t, wvT.rearrange("(dc p) e -> p dc e", p=128))
            nc.sync.dma_start(wo_t, woT.rearrange("(ec p) o -> p ec o", p=128))
            idn_t = wpool.tile([128, 128], BF16, tag="idn")
            nc.sync.dma_start(idn_t, idn.rearrange("a b -> a b"))
            pm_t = wpool.tile([128, 32], F32, tag="pm")
            nc.sync.dma_start(pm_t, pmt_r)
            ones_t = wpool.tile([128, 128], BF16, tag="ones")
            nc.vector.memset(ones_t, 1.0)

            # persistent block-diagonal Q (off-parity blocks stay zero)
            bdqs = [
                persist.tile([128, 4, 8, 64], BF16, tag=f"bdq{i}", name=f"bdq{i}")
                for i in range(2)
            ]
            for t in bdqs:
                nc.vector.memset(t, 0.0)

            xts = {}
            kfs = {}
            mns = {}

            def load_xt(g):
                xt = xtp.tile([128, 4, GB * NE], BF16, tag="xt", name=f"xt{g}")
                t0 = g * GB * NE
                for h in range(2):
                    nc.sync.dma_start(
                        xt[:, :, h * 512 : (h + 1) * 512],
                        ent_r[:, :, t0 + h * 512 : t0 + (h + 1) * 512],
                    )
                xts[g] = xt

            def load_mn(g):
                mn = mnp.tile([128, 8, 64], BF16, tag="mn", name=f"mn{g}")
                nc.sync.dma_start(
                    mn, mneg.rearrange("p (g x) -> p g x", g=NG)[:, g, :]
                )
                mns[g] = mn

            def kproj(g):
                """K projection for group g -> kf[g] (bf16)."""
                xt = xts[g]
                kf = kfp.tile([128, 4, GB * NE], BF16, tag="kf", name=f"kf{g}")
                for ec in range(4):
                    for fg in range(2):
                        ps_k = psp.tile([128, 512], F32, tag="ps", name="ps_k")
                        for dc in range(4):
                            nc.tensor.matmul(
                                ps_k,
                                wk_t[:, dc, ec * 128 : (ec + 1) * 128],
                                xt[:, dc, fg * 512 : (fg + 1) * 512],
                                start=(dc == 0), stop=(dc == 3),
                            )
                        nc.scalar.copy(kf[:, ec, fg * 512 : (fg + 1) * 512], ps_k)
                kfs[g] = kf

            def qproj(g):
                """Q projection for group g -> block-diagonal bdq[g%2]."""
                xt = xts[g]
                bdq = bdqs[g % 2]
                xq_view = xt.rearrange("p dc (b t) -> p dc b t", b=GB)
                ps_q = psqp.tile([128, 4, 256], F32, tag="psq", name="ps_q")
                for ec in range(4):
                    for dc in range(4):
                        nc.tensor.matmul(
                            ps_q[:, ec, :],
                            wq_t[:, dc, ec * 128 : (ec + 1) * 128],
                            xq_view[:, dc, :, 0:NQ],
                            start=(dc == 0), stop=(dc == 3),
                        )
                ps_qv = ps_q.rearrange("p c (g2 x) -> p c g2 x", g2=8)
                nc.scalar.copy(bdq[0:64, :, :, 0:32], ps_qv[0:64])
                nc.scalar.copy(bdq[64:128, :, :, 32:64], ps_qv[64:128])

            # ---- prologue: group 0's K/Q + first mask ----
            load_xt(0)
            load_mn(0)
            kproj(0)
            qproj(0)

            for g in range(NG):
                xt = xts.pop(g)
                kf = kfs.pop(g)
                mn = mns.pop(g)
                bdq = bdqs[g % 2]

                # ---- logits + mask + exp -> bde ----
                bde = bdep.tile([128, 4, 8, 64], BF16, tag="bde", name=f"bde{g}")
                for c in range(4):
                    ps_l = pslp.tile([128, 8, 64], F32, tag="psl", name="ps_l")
                    nc.tensor.matmul(
                        ps_l.rearrange("p a b -> p (a b)"),
                        idn_t,
                        mn.rearrange("p a b -> p (a b)"),
                        start=True, stop=False,
                        skip_group_check=True,
                    )
                    for g2 in range(8):
                        nc.tensor.matmul(
                            ps_l[:, g2, :],
                            kf[:, c, g2 * 128 : (g2 + 1) * 128],
                            bdq[:, c, g2, :],
                            start=False, stop=(g2 == 7),
                            skip_group_check=True,
                        )
                    nc.scalar.activation(
                        bde[:, c, :, :], ps_l, AF.Exp, scale=1.0 / np.sqrt(HD)
                    )

                if debug and g == 0:
                    nc.sync.dma_start(dbg["dxt"].rearrange("a b -> a b"), xt.rearrange("p a b -> p (a b)"))
                    nc.sync.dma_start(dbg["dkf"].rearrange("a b -> a b"), kf.rearrange("p a b -> p (a b)"))
                    nc.sync.dma_start(dbg["dbdq"].rearrange("a b -> a b"), bdq.rearrange("p a b c -> p (a b c)"))
                    nc.sync.dma_start(dbg["dbde"].rearrange("a b -> a b"), bde.rearrange("p a b c -> p (a b c)"))

                # ---- V projection (overlaps exp on ACT) ----
                vt = vtp.tile([128, 8, 512], BF16, tag="vt", name=f"vt{g}")
                for t8 in range(8):
                    ps_v = psp.tile([128, 512], F32, tag="ps", name="ps_v")
                    for dc in range(4):
                        nc.tensor.matmul(
                            ps_v,
                            xt[:, dc, t8 * 128 : (t8 + 1) * 128],
                            wv_t[:, dc, :],
                            start=(dc == 0), stop=(dc == 3),
                        )
                    nc.vector.tensor_copy(vt[:, t8, :], ps_v)

                # ---- softmax denominators (replicated across partitions) ----
                rs = rsp.tile([128, 4, 512], F32, tag="rs", name=f"rs{g}")
                for c in range(4):
                    ps_s = pslp.tile([128, 512], F32, tag="psl", name="ps_s")
                    nc.tensor.matmul(
                        ps_s,
                        ones_t,
                        bde[:, c, :, :].rearrange("p a b -> p (a b)"),
                        start=True, stop=True,
                    )
                    nc.vector.reciprocal_approx_fast(out=rs[:, c, :], in_=ps_s)

                if debug and g == 0:
                    nc.sync.dma_start(dbg["drs"].rearrange("a b -> a b"), rs.rearrange("p a b -> p (a b)"))
                    nc.sync.dma_start(dbg["dvt"].rearrange("a b -> a b"), vt.rearrange("p a b -> p (a b)"))

                # ---- next group's K/Q keep the PE busy while DVE/ACT drain ----
                if g + 1 < NG:
                    load_xt(g + 1)
                    load_mn(g + 1)
                    kproj(g + 1)
                    qproj(g + 1)

                # ---- attn @ V with junk parity blocks; extract diag * rs ----
                ao = aop.tile([128, 4, 256], BF16, tag="ao", name=f"ao{g}")
                ao_v = ao.rearrange("p c (h2 gi x) -> p c h2 gi x", h2=2, gi=4)
                rs_v = rs.rearrange("p c (g2 h2 x) -> p c g2 h2 x", g2=8, h2=2)
                for half in range(2):
                    av = avp.tile([128, 4, 4, 64], F32, tag="av", name="ps_av")
                    for gi in range(4):
                        g2 = half * 4 + gi
                        for c in range(4):
                            nc.tensor.matmul(
                                av[:, gi, c, :],
                                vt[:, g2, c * 128 : (c + 1) * 128],
                                bde[:, c, g2, :],
                                start=True, stop=True,
                            )
                    av_v = av.rearrange("p gi c x -> p c gi x")
                    for P in range(2):
                        psl = slice(P * 64, (P + 1) * 64)
                        nc.vector.tensor_tensor(
                            ao_v[psl, :, half, :, :],
                            av_v[psl, :, :, P * 32 : (P + 1) * 32],
                            rs_v[psl, :, half * 4 : (half + 1) * 4, P, :],
                            ALU.mult,
                        )

                if debug and g == 0:
                    nc.sync.dma_start(dbg["dao"].rearrange("a b -> a b"), ao.rearrange("p a b -> p (a b)"))

                # ---- output projection + post-mask + store ----
                for tc2 in range(2):
                    ps_o = psp.tile([128, 512], F32, tag="ps", name="ps_o")
                    for ec in range(4):
                        nc.tensor.matmul(
                            ps_o,
                            ao[:, ec, tc2 * 128 : (tc2 + 1) * 128],
                            wo_t[:, ec, :],
                            start=(ec == 0), stop=(ec == 3),
                        )
                    out_s = osp.tile([128, 512], F32, tag="outs", name="out_s")
                    nc.scalar.activation(
                        out_s, ps_o, AF.Copy,
                        scale=pm_t[:, g * 2 + tc2 : g * 2 + tc2 + 1],
                    )
                    q0 = g * GB * NQ
                    nc.sync.dma_start(
                        out[q0 + tc2 * 128 : q0 + (tc2 + 1) * 128, :], out_s
                    )

    nc.finalize()
    return nc


_NC_CACHE = None
RUN_KWARGS = {}
LAST_RESULT = None


def _get_nc():
    global _NC_CACHE
    if _NC_CACHE is None:
        _NC_CACHE = build_nc()
    return _NC_CACHE


def _bf16(x):
    return np.ascontiguousarray(x.astype(ml_dtypes.bfloat16))


def kernel(entities, pre_mask, post_mask, W_in, W_out, b_out):
    entities = np.asarray(entities, dtype=np.float32)
    pre_mask = np.asarray(pre_mask)
    post_mask = np.asarray(post_mask)
    W_in = np.asarray(W_in, dtype=np.float32)
    W_out = np.asarray(W_out, dtype=np.float32)
    b_out = np.asarray(b_out, dtype=np.float32)

    wqT = _bf16(W_in[0:512].T)
    wkT = _bf16(W_in[512:1024].T)
    wvT = _bf16(W_in[1024:1536].T)
    woT = _bf16(W_out.T)
    idn = _bf16(np.eye(128, dtype=np.float32))

    bp_idx = np.arange(2).reshape(2, 1, 1, 1, 1, 1, 1)
    B_idx = np.arange(2).reshape(1, 1, 1, 1, 1, 2, 1)

    in_maps = []
    for i in range(NCORES):
        bsl = slice(i * BPC, (i + 1) * BPC)
        ent_i = _bf16(entities[bsl].reshape(NTOK, D).T)
        # mneg[(bp,j), (g,g2,P,B,q)]: -1e30 where cross-batch or pre-masked
        pm_i = pre_mask[bsl, :NQ, :]                       # (256, 16, 64)
        pm_r = pm_i.reshape(NG, 8, 2, NQ, NE)              # (g, g2, B, q, j)
        pmx = pm_r.transpose(4, 0, 1, 2, 3)                # (j, g, g2, B, q)
        cond = bp_idx != B_idx                             # (2,1,1,1,1,2,1)
        cond = cond | pmx[None, :, :, :, None, :, :]       # (2,j,g,g2,P,B,q)
        cond = np.broadcast_to(cond, (2, NE, NG, 8, 2, 2, NQ))
        mneg_i = _bf16(np.where(cond, -1e30, 0.0).reshape(128, NG * 512))
        pmt_i = np.ascontiguousarray(
            (1.0 - post_mask[bsl].astype(np.float32)).reshape(NQT)
        )
        in_maps.append(
            {
                "ent": ent_i,
                "mneg": mneg_i,
                "pmt": pmt_i,
                "wqT": wqT,
                "wkT": wkT,
                "wvT": wvT,
                "woT": woT,
                "idn": idn,
            }
        )

    nc = _get_nc()
    res = bass_utils.run_bass_kernel_spmd(
        nc, in_maps, list(range(NCORES)), **RUN_KWARGS
    )
    global LAST_RESULT
    LAST_RESULT = res
    outs = [res.results[i]["out"].reshape(BPC, NQ, 512) for i in range(NCORES)]
    full = np.concatenate(outs, axis=0)
    if b_out.any():
        full = full + b_out[None, None, :]
        full = np.where(post_mask[:, :, None], 0.0, full)
    return full.astype(np.float32)


# revision 13
# speedup vs baseline: 3.2008x; 1.0077x over previous
"""EntityAttentionLayer Trainium2 kernel (v2, all-bf16 datapath).

Data-parallel over batch across 8 NeuronCores (256 batches/core).
Per core, per G-group of 16 batches (1024 entity tokens, 256 query tokens),
software-pipelined so group g's attention runs while group g+1's K/Q
projections keep the PE busy:

  xt:    entities, feature-on-partition, bf16        [128, 4dc, 1024]
  kf:    K feature-major GEMM -> bf16                [128, 4c, 1024]
  vt:    V token-major GEMM -> bf16                  [128, 8t, 512]
  bdq:   Q gathered from xt via strided moving AP, written block-diagonal
         over head-parity                            [128, 4c, 8g2, 64]
  logits: per (c, g2): kf[128,128] stationary (2 batches) x bdq cols
         -> psum [128 tok, 8g2, 64]; junk cross-batch cells + pre-mask
         handled by ONE identity-stationary matmul adding mneg (-1e30)
  exp:   ACT -> bde bf16 (masked cells exp -> 0)     [128, 4c, 8g2, 64]
  sums:  all-ones [128,128] stationary matmul -> denominators replicated
         across all 128 partitions; reciprocal_approx_fast -> rs f32
  attnv: vt stationary x bde moving -> psum with junk parity blocks;
         diagonal blocks extracted * rs (fused normalize) -> ao bf16
  oproj: ao stationary x woT moving; post-mask applied via ACT Copy with
         per-partition scale; DMA out f32
"""

import os
import sys

import numpy as np

sys.path.insert(0, "/opt/trn_rl_repo")

import concourse.bass as bass
import concourse.mybir as mybir
import concourse.tile as tile
from concourse import bacc, bass_utils

import ml_dtypes

F32 = mybir.dt.float32
BF16 = mybir.dt.bfloat16
AF = mybir.ActivationFunctionType
ALU = mybir.AluOpType

BS, NE, NQ = 2048, 64, 16
D = 512
H = 8
HD = 64
NCORES = 8
BPC = BS // NCORES          # 256 batches per core
GB = 16                     # batches per G-group
NG = BPC // GB              # 16 groups
NTOK = BPC * NE             # 16384 entity tokens per core
NQT = BPC * NQ              # 4096 query tokens per core


def build_nc(debug=False):
    nc = bacc.Bacc()
    dbg = {}
    if debug:
        for nm, shp, dt in [
            ("dxt", [128, 4096], BF16), ("dkf", [128, 4096], BF16),
            ("dbdq", [128, 2048], BF16), ("dbde", [128, 2048], BF16),
            ("dvt", [128, 4096], BF16), ("drs", [128, 2048], F32),
            ("dao", [128, 1024], BF16),
        ]:
            dbg[nm] = nc.declare_dram_parameter(nm, shp, dt, isOutput=True)

    ent = nc.declare_dram_parameter("ent", [D, NTOK], BF16, isOutput=False)
    msk = nc.declare_dram_parameter("msk", [128, NG * 512], BF16, isOutput=False)
    pmt = nc.declare_dram_parameter("pmt", [NQT], F32, isOutput=False)
    wqT = nc.declare_dram_parameter("wqT", [D, 512], BF16, isOutput=False)
    wkT = nc.declare_dram_parameter("wkT", [D, 512], BF16, isOutput=False)
    wvT = nc.declare_dram_parameter("wvT", [D, 512], BF16, isOutput=False)
    woT = nc.declare_dram_parameter("woT", [512, 512], BF16, isOutput=False)
    out = nc.declare_dram_parameter("out", [NQT, 512], F32, isOutput=True)

    ent_r = ent.rearrange("(dc p) n -> p dc n", p=128)   # [128, 4, 16384]
    pmt_r = pmt.rearrange("(o p) -> p o", p=128)         # [128, 32]

    with tile.TileContext(nc) as tc:
        with (
            tc.tile_pool(name="wpool", bufs=1) as wpool,
            tc.tile_pool(name="xtp", bufs=2) as xtp,
            tc.tile_pool(name="kfp", bufs=2) as kfp,
            tc.tile_pool(name="vtp", bufs=2) as vtp,
            tc.tile_pool(name="persist", bufs=1) as persist,
            tc.tile_pool(name="bdep", bufs=2) as bdep,
            tc.tile_pool(name="mnp", bufs=2) as mnp,
            tc.tile_pool(name="rsp", bufs=2) as rsp,
            tc.tile_pool(name="aop", bufs=2) as aop,
            tc.tile_pool(name="osp", bufs=2) as osp,
            tc.tile_pool(name="psp", bufs=2, space="PSUM") as psp,
            tc.tile_pool(name="pslp", bufs=2, space="PSUM") as pslp,
            tc.tile_pool(name="avp", bufs=2, space="PSUM") as avp,
        ):
            # ---- constants / weights (loaded once) ----
            wq_t = wpool.tile([128, 4, 512], BF16, tag="wq")
            wk_t = wpool.tile([128, 4, 512], BF16, tag="wk")
            wv_t = wpool.tile([128, 4, 512], BF16, tag="wv")
            wo_t = wpool.tile([128, 4, 512], BF16, tag="wo")
            nc.sync.dma_start(wk_t, wkT.rearrange("(dc p) e -> p dc e", p=128))
            nc.sync.dma_start(wq_t, wqT.rearrange("(dc p) e -> p dc e", p=128))
            nc.sync.dma_start(wv_t, wvT.rearrange("(dc p) e -> p dc e", p=128))
            nc.sync.dma_start(wo_t, woT.rearrange("(ec p) o -> p ec o", p=128))
            pm_t = wpool.tile([128, 32], F32, tag="pm")
            nc.sync.dma_start(pm_t, pmt_r)
            ones_t = wpool.tile([128, 128], BF16, tag="ones")
            nc.vector.memset(ones_t, 1.0)

            # persistent block-diagonal Q (off-parity blocks stay zero)
            bdqs = [
                persist.tile([128, 4, 8, 64], BF16, tag=f"bdq{i}", name=f"bdq{i}")
                for i in range(2)
            ]
            for t in bdqs:
                nc.vector.memset(t, 0.0)

            xts = {}
            kfs = {}
            mns = {}
            pending_o = []

            def oproj_flush():
                while pending_o:
                    og, oao = pending_o.pop(0)
                    for tc2 in range(2):
                        ps_o = psp.tile([128, 512], F32, tag="ps", name="ps_o")
                        for ec in range(4):
                            nc.tensor.matmul(
                                ps_o,
                                oao[:, ec, tc2 * 128 : (tc2 + 1) * 128],
                                wo_t[:, ec, :],
                                start=(ec == 0), stop=(ec == 3),
                            )
                        out_s = osp.tile([128, 512], F32, tag="outs", name="out_s")
                        nc.scalar.activation(
                            out_s, ps_o, AF.Copy,
                            scale=pm_t[:, og * 2 + tc2 : og * 2 + tc2 + 1],
                        )
                        q0 = og * GB * NQ
                        nc.sync.dma_start(
                            out[q0 + tc2 * 128 : q0 + (tc2 + 1) * 128, :], out_s
                        )

            def load_xt(g):
                xt = xtp.tile([128, 4, GB * NE], BF16, tag="xt", name=f"xt{g}")
                t0 = g * GB * NE
                for h in range(2):
                    nc.sync.dma_start(
                        xt[:, :, h * 512 : (h + 1) * 512],
                        ent_r[:, :, t0 + h * 512 : t0 + (h + 1) * 512],
                    )
                xts[g] = xt

            def load_mn(g):
                mn = mnp.tile([128, 8, 64], BF16, tag="mn", name=f"mn{g}")
                nc.sync.dma_start(
                    mn, msk.rearrange("p (g x) -> p g x", g=NG)[:, g, :]
                )
                mns[g] = mn

            def kproj(g):
                """K projection for group g -> kf[g] (bf16)."""
                xt = xts[g]
                kf = kfp.tile([128, 4, GB * NE], BF16, tag="kf", name=f"kf{g}")
                for ec in range(4):
                    for fg in range(2):
                        ps_k = psp.tile([128, 512], F32, tag="ps", name="ps_k")
                        for dc in range(4):
                            nc.tensor.matmul(
                                ps_k,
                                wk_t[:, dc, ec * 128 : (ec + 1) * 128],
                                xt[:, dc, fg * 512 : (fg + 1) * 512],
                                start=(dc == 0), stop=(dc == 3),
                            )
                        nc.scalar.copy(kf[:, ec, fg * 512 : (fg + 1) * 512], ps_k)
                kfs[g] = kf

            def qproj(g):
                """Q projection for group g -> block-diagonal bdq[g%2]."""
                xt = xts[g]
                bdq = bdqs[g % 2]
                xq_view = xt.rearrange("p dc (b t) -> p dc b t", b=GB)
                for eh in range(2):
                    ps_q = psp.tile([128, 2, 256], F32, tag="ps", name="ps_q")
                    for ei in range(2):
                        ec = eh * 2 + ei
                        for dc in range(4):
                            nc.tensor.matmul(
                                ps_q[:, ei, :],
                                wq_t[:, dc, ec * 128 : (ec + 1) * 128],
                                xq_view[:, dc, :, 0:NQ],
                                start=(dc == 0), stop=(dc == 3),
                            )
                    ps_qv = ps_q.rearrange("p c (g2 x) -> p c g2 x", g2=8)
                    cs = slice(eh * 2, eh * 2 + 2)
                    nc.scalar.copy(bdq[0:64, cs, :, 0:32], ps_qv[0:64])
                    nc.scalar.copy(bdq[64:128, cs, :, 32:64], ps_qv[64:128])

            # ---- prologue: group 0's K/Q + first mask ----
            load_xt(0)
            load_mn(0)
            kproj(0)
            qproj(0)

            for g in range(NG):
                xt = xts.pop(g)
                kf = kfs.pop(g)
                mn = mns.pop(g)
                bdq = bdqs[g % 2]

                # ---- logits + mask + exp -> bde ----
                bde = bdep.tile([128, 4, 8, 64], BF16, tag="bde", name=f"bde{g}")
                for c in range(4):
                    ps_l = pslp.tile([128, 8, 64], F32, tag="psl", name="ps_l")
                    for g2 in range(8):
                        nc.tensor.matmul(
                            ps_l[:, g2, :],
                            kf[:, c, g2 * 128 : (g2 + 1) * 128],
                            bdq[:, c, g2, :],
                            start=True, stop=True,
                        )
                    nc.scalar.activation(
                        bde[:, c, :, :], ps_l, AF.Exp, scale=1.0 / np.sqrt(HD)
                    )
                    # zero masked + cross-batch junk cells on the idle Pool engine
                    nc.gpsimd.tensor_tensor(
                        bde[:, c, :, :], bde[:, c, :, :], mn, ALU.mult
                    )

                if debug and g == 0:
                    nc.sync.dma_start(dbg["dxt"].rearrange("a b -> a b"), xt.rearrange("p a b -> p (a b)"))
                    nc.sync.dma_start(dbg["dkf"].rearrange("a b -> a b"), kf.rearrange("p a b -> p (a b)"))
                    nc.sync.dma_start(dbg["dbdq"].rearrange("a b -> a b"), bdq.rearrange("p a b c -> p (a b c)"))
                    nc.sync.dma_start(dbg["dbde"].rearrange("a b -> a b"), bde.rearrange("p a b c -> p (a b c)"))

                # ---- previous group's output projection (covers extract) ----
                oproj_flush()

                # ---- V projection (overlaps exp on ACT) ----
                vt = vtp.tile([128, 8, 512], BF16, tag="vt", name=f"vt{g}")
                for t8 in range(8):
                    ps_v = psp.tile([128, 512], F32, tag="ps", name="ps_v")
                    for dc in range(4):
                        nc.tensor.matmul(
                            ps_v,
                            xt[:, dc, t8 * 128 : (t8 + 1) * 128],
                            wv_t[:, dc, :],
                            start=(dc == 0), stop=(dc == 3),
                        )
                    if t8 < 4:
                        nc.scalar.copy(vt[:, t8, :], ps_v)
                    else:
                        nc.vector.tensor_copy(vt[:, t8, :], ps_v)

                # ---- softmax denominators (replicated across partitions) ----
                rs = rsp.tile([128, 4, 512], F32, tag="rs", name=f"rs{g}")
                for c in range(4):
                    ps_s = pslp.tile([128, 512], F32, tag="psl", name="ps_s")
                    nc.tensor.matmul(
                        ps_s,
                        ones_t,
                        bde[:, c, :, :].rearrange("p a b -> p (a b)"),
                        start=True, stop=True,
                    )
                    nc.vector.reciprocal_approx_fast(out=rs[:, c, :], in_=ps_s)

                if debug and g == 0:
                    nc.sync.dma_start(dbg["drs"].rearrange("a b -> a b"), rs.rearrange("p a b -> p (a b)"))
                    nc.sync.dma_start(dbg["dvt"].rearrange("a b -> a b"), vt.rearrange("p a b -> p (a b)"))

                # ---- next group's K/Q keep the PE busy while DVE/ACT drain ----
                if g + 1 < NG:
                    load_xt(g + 1)
                    load_mn(g + 1)
                    kproj(g + 1)
                    qproj(g + 1)

                # ---- attn @ V with junk parity blocks; extract diag * rs ----
                ao = aop.tile([128, 4, 256], BF16, tag="ao", name=f"ao{g}")
                ao_v = ao.rearrange("p c (h2 gi x) -> p c h2 gi x", h2=2, gi=4)
                rs_v = rs.rearrange("p c (g2 h2 x) -> p c g2 h2 x", g2=8, h2=2)
                for half in range(2):
                    av = avp.tile([128, 4, 4, 64], F32, tag="av", name="ps_av")
                    for gi in range(4):
                        g2 = half * 4 + gi
                        for c in range(4):
                            nc.tensor.matmul(
                                av[:, gi, c, :],
                                vt[:, g2, c * 128 : (c + 1) * 128],
                                bde[:, c, g2, :],
                                start=True, stop=True,
                            )
                    av_v = av.rearrange("p gi c x -> p c gi x")
                    for P in range(2):
                        psl = slice(P * 64, (P + 1) * 64)
                        nc.vector.tensor_tensor(
                            ao_v[psl, :, half, :, :],
                            av_v[psl, :, :, P * 32 : (P + 1) * 32],
                            rs_v[psl, :, half * 4 : (half + 1) * 4, P, :],
                            ALU.mult,
                        )

                if debug and g == 0:
                    nc.sync.dma_start(dbg["dao"].rearrange("a b -> a b"), ao.rearrange("p a b -> p (a b)"))

                pending_o.append((g, ao))

            oproj_flush()

    nc.finalize()
    return nc


_NC_CACHE = None
RUN_KWARGS = {}
LAST_RESULT = None


def _get_nc():
    global _NC_CACHE
    if _NC_CACHE is None:
        _NC_CACHE = build_nc()
    return _NC_CACHE


def _bf16(x):
    return np.ascontiguousarray(x.astype(ml_dtypes.bfloat16))


def kernel(entities, pre_mask, post_mask, W_in, W_out, b_out):
    entities = np.asarray(entities, dtype=np.float32)
    pre_mask = np.asarray(pre_mask)
    post_mask = np.asarray(post_mask)
    W_in = np.asarray(W_in, dtype=np.float32)
    W_out = np.asarray(W_out, dtype=np.float32)
    b_out = np.asarray(b_out, dtype=np.float32)

    wqT = _bf16(W_in[0:512].T)
    wkT = _bf16(W_in[512:1024].T)
    wvT = _bf16(W_in[1024:1536].T)
    woT = _bf16(W_out.T)

    bp_idx = np.arange(2).reshape(2, 1, 1, 1, 1, 1, 1)
    B_idx = np.arange(2).reshape(1, 1, 1, 1, 1, 2, 1)

    in_maps = []
    for i in range(NCORES):
        bsl = slice(i * BPC, (i + 1) * BPC)
        ent_i = _bf16(entities[bsl].reshape(NTOK, D).T)
        # mneg[(bp,j), (g,g2,P,B,q)]: -1e30 where cross-batch or pre-masked
        pm_i = pre_mask[bsl, :NQ, :]                       # (256, 16, 64)
        pm_r = pm_i.reshape(NG, 8, 2, NQ, NE)              # (g, g2, B, q, j)
        pmx = pm_r.transpose(4, 0, 1, 2, 3)                # (j, g, g2, B, q)
        cond = bp_idx != B_idx                             # (2,1,1,1,1,2,1)
        cond = cond | pmx[None, :, :, :, None, :, :]       # (2,j,g,g2,P,B,q)
        cond = np.broadcast_to(cond, (2, NE, NG, 8, 2, 2, NQ))
        msk_i = _bf16(np.where(cond, 0.0, 1.0).reshape(128, NG * 512))
        pmt_i = np.ascontiguousarray(
            (1.0 - post_mask[bsl].astype(np.float32)).reshape(NQT)
        )
        in_maps.append(
            {
                "ent": ent_i,
                "msk": msk_i,
                "pmt": pmt_i,
                "wqT": wqT,
                "wkT": wkT,
                "wvT": wvT,
                "woT": woT,
            }
        )

    nc = _get_nc()
    res = bass_utils.run_bass_kernel_spmd(
        nc, in_maps, list(range(NCORES)), **RUN_KWARGS
    )
    global LAST_RESULT
    LAST_RESULT = res
    outs = [res.results[i]["out"].reshape(BPC, NQ, 512) for i in range(NCORES)]
    full = np.concatenate(outs, axis=0)
    if b_out.any():
        full = full + b_out[None, None, :]
        full = np.where(post_mask[:, :, None], 0.0, full)
    return full.astype(np.float32)
